# revision 2
# baseline (speedup 1.0000x reference)
"""Blur (UpFirDn2D up=2, k=outer([1,3,3,1])/16*4, dn=2) kernel for 8 trn2 NeuronCores.

Math: the up-fir-dn with these factors collapses to a 2x2 stencil
    y[c,i,j] = (9*x[c,i,j] + 3*x[c,i,j+1] + 3*x[c,i+1,j] + 1*x[c,i+1,j+1]) / 16
(zero padding past the high edge). Separable: taps [3,1]/4 per axis.

Strategy (per core, pure batch-parallel: core b handles x[b]):
  - Layout: H=128 on SBUF partitions, free dim = (channel, W).
  - Load chunks of channels with an fp32->bf16 cast during DMA (SWDGE).
  - Whole stencil on the TensorEngine as 2 accumulating matmuls per 512-col
    block:  psum = W1.T @ x + W2.T @ shift_w(x)
    where W1 = (3*S)^T, W2 = S^T and S is the banded vertical-tap matrix
    (diag 3/16, superdiag 1/16).  shift_w is just an AP offset; the w=127
    column boundary is handled by restricting the second matmul to w<127.
    All tap weights (9/16, 3/16, 1/16) are bf16-exact; PSUM accumulates fp32,
    so the only error is the bf16 rounding of x (~2e-3 relative).
  - PSUM -> SBUF fp32 copies alternate between ScalarE and VectorE.
  - fp32 store via HWDGE.
"""

import sys

sys.path.insert(0, "/opt/trn_rl_repo")

import numpy as np
import ml_dtypes

import concourse.bacc as bacc
import concourse.mybir as mybir
from concourse.tile import TileContext
from concourse.bass_utils import run_bass_kernel_spmd

B, C, H, W = 8, 256, 128, 128
NCH = 16  # channels per chunk
N_CORES = 8

_cache = {}


def _weights():
    s = np.zeros((128, 128), dtype=np.float32)
    idx = np.arange(128)
    s[idx, idx] = 3.0 / 16.0
    s[idx[:-1], idx[:-1] + 1] = 1.0 / 16.0
    # lhsT convention: out = lhsT.T @ rhs, want out = S @ rhs -> lhsT = S.T
    w2 = s.T.astype(ml_dtypes.bfloat16)
    w1 = (3.0 * s.T).astype(ml_dtypes.bfloat16)
    return w1, w2


def _build():
    nc = bacc.Bacc("TRN2", target_bir_lowering=False, debug=False,
                   num_devices=N_CORES)
    f32 = mybir.dt.float32
    bf16 = mybir.dt.bfloat16
    x = nc.dram_tensor("x", [C, H, W], f32, kind="ExternalInput")
    w1 = nc.dram_tensor("w1", [128, 128], bf16, kind="ExternalInput")
    w2 = nc.dram_tensor("w2", [128, 128], bf16, kind="ExternalInput")
    y = nc.dram_tensor("y", [C, H, W], f32, kind="ExternalOutput")

    with TileContext(nc) as tc:
        with (
            tc.tile_pool(name="wpool", bufs=1) as wpool,
            tc.tile_pool(name="xpool", bufs=3) as xpool,
            tc.tile_pool(name="ypool", bufs=3) as ypool,
            tc.tile_pool(name="psum", bufs=2, space="PSUM") as pp,
        ):
            w1_sb = wpool.tile([128, 128], bf16, tag="w1")
            w2_sb = wpool.tile([128, 128], bf16, tag="w2")
            nc.sync.dma_start(out=w1_sb[:], in_=w1[:])
            nc.sync.dma_start(out=w2_sb[:], in_=w2[:])

            copy_toggle = 0
            for ci in range(0, C, NCH):
                xt = xpool.tile([128, NCH, W], bf16)
                nc.gpsimd.dma_start(
                    out=xt[:], in_=x[ci : ci + NCH].rearrange("c h w -> h c w")
                )
                ps = pp.tile([128, NCH, W], f32)
                for b in range(NCH * W // 512):
                    cs = b * (512 // W)  # channels per 512-col block
                    ce = cs + (512 // W)
                    nc.tensor.matmul(
                        ps[:, cs:ce, :], w1_sb[:], xt[:, cs:ce, :],
                        start=True, stop=False,
                    )
                    for c in range(cs, ce):
                        nc.tensor.matmul(
                            ps[:, c, 0 : W - 1], w2_sb[:], xt[:, c, 1:W],
                            start=False, stop=(c == ce - 1),
                        )
                yt = ypool.tile([128, NCH, W], f32)
                if copy_toggle == 0:
                    nc.scalar.copy(yt[:], ps[:])
                else:
                    nc.vector.tensor_copy(out=yt[:], in_=ps[:])
                copy_toggle ^= 1
                nc.sync.dma_start(
                    out=y[ci : ci + NCH].rearrange("c h w -> h c w"), in_=yt[:]
                )
    nc.compile()
    return nc


def kernel(x: np.ndarray) -> np.ndarray:
    assert x.shape == (B, C, H, W) and x.dtype == np.float32
    if "nc" not in _cache:
        _cache["nc"] = _build()
    nc = _cache["nc"]
    w1, w2 = _weights()
    in_maps = [
        {"x": np.ascontiguousarray(x[i]), "w1": w1, "w2": w2} for i in range(B)
    ]
    res = run_bass_kernel_spmd(nc, in_maps, core_ids=list(range(N_CORES)))
    return np.stack([res.results[i]["y"] for i in range(N_CORES)], axis=0)


if __name__ == "__main__":
    rng = np.random.default_rng(0)
    x = rng.standard_normal((B, C, H, W), dtype=np.float32)
    y = kernel(x)
    xp = np.pad(x, ((0, 0), (0, 0), (0, 1), (0, 1)))
    ref = (
        9 * xp[:, :, :-1, :-1] + 3 * xp[:, :, :-1, 1:]
        + 3 * xp[:, :, 1:, :-1] + 1 * xp[:, :, 1:, 1:]
    ) / 16
    err = np.abs(y - ref).max()
    print("absmax err vs numpy stencil:", err, "scale:", np.abs(ref).max())


# revision 3
# speedup vs baseline: 1.0641x; 1.0641x over previous
"""Blur (UpFirDn2D, up=2, blur_kernel=outer([1,3,3,1]) normalized * 4, dn=2)
for 8 Trainium2 NeuronCores, data-parallel over the batch dimension.

Math: with these factors the up->FIR->down pipeline collapses exactly to a
2x2 stencil with separable taps [3,1]/4 per axis:

    y[b,c,i,j] = (9*x[b,c,i,j] + 3*x[b,c,i,j+1]
                  + 3*x[b,c,i+1,j] + 1*x[b,c,i+1,j+1]) / 16

(out-of-range taps are zero).  Verified bit-close (absmax ~5e-7) against
lax.conv_general_dilated with lhs_dilation=2, stride=2, pad=((1,2),(1,2)).

Per-core kernel (one batch element: 256 channels of 128x128, fp32):
  - SBUF partitions = 128 channels (two channel halves); free dim = rows of
    an h-strip x W.  Channel planes are contiguous in DRAM, so every DMA
    descriptor moves (STRIP+1)*512B contiguous bytes -> line-rate HBM DMA,
    no dtype casts, HWDGE both directions.
  - h-strips of 16 rows plus a 1-row halo.
  - compute (exact fp32):
      p = x * (1/16)                      ScalarE, in-place (exact: /2^4)
      t = 3*p + shift_w(p)                VectorE scalar_tensor_tensor
      y = 3*t + shift_h(t)                VectorE scalar_tensor_tensor
    Both shifts are free-dim AP offsets; image-edge columns/rows get the
    2-tap-only value via small tensor_scalar ops.
  - Tile framework handles all semaphores/double-buffering (x:4, t:3, y:4
    buffers), overlapping DMA in / ScalarE / VectorE / DMA out across strips.

Measured on trn2: ~102 us/core HW exec (HBM roofline for 16 MiB in + 16 MiB
out is ~94 us), absmax error vs the fp32 reference ~5e-7.
"""

import sys

if "/opt/trn_rl_repo" not in sys.path:
    sys.path.insert(0, "/opt/trn_rl_repo")

import numpy as np

import concourse.bacc as bacc
import concourse.mybir as mybir
from concourse.tile import TileContext
from concourse.bass_utils import run_bass_kernel_spmd

B, C, H, W = 8, 256, 128, 128
N_CORES = 8
STRIP = 16

_cache = {}


def _build():
    nc = bacc.Bacc("TRN2", target_bir_lowering=False, debug=False,
                   num_devices=N_CORES)
    f32 = mybir.dt.float32
    mult = mybir.AluOpType.mult
    add = mybir.AluOpType.add
    x = nc.dram_tensor("x", [C, H, W], f32, kind="ExternalInput")
    y = nc.dram_tensor("y", [C, H, W], f32, kind="ExternalOutput")

    with TileContext(nc) as tc:
        with (
            tc.tile_pool(name="xpool", bufs=4) as xpool,
            tc.tile_pool(name="tpool", bufs=3) as tpool,
            tc.tile_pool(name="ypool", bufs=4) as ypool,
        ):
            for c0 in range(0, C, 128):
                for h0 in range(0, H, STRIP):
                    last = h0 + STRIP >= H
                    hr = STRIP if last else STRIP + 1  # rows incl. halo
                    xt = xpool.tile([128, STRIP + 1, W], f32, tag="x")
                    nc.sync.dma_start(
                        out=xt[:, :hr, :], in_=x[c0 : c0 + 128, h0 : h0 + hr, :]
                    )
                    # p = x/16 in-place (exact power-of-two scale)
                    nc.scalar.mul(xt[:, :hr, :], xt[:, :hr, :], 1.0 / 16.0)
                    # t = 3p + shift_w(p); last image column: t = 3p
                    tt = tpool.tile([128, STRIP + 1, W], f32, tag="t")
                    nc.vector.scalar_tensor_tensor(
                        out=tt[:, :hr, 0 : W - 1],
                        in0=xt[:, :hr, 0 : W - 1],
                        scalar=3.0,
                        in1=xt[:, :hr, 1:W],
                        op0=mult,
                        op1=add,
                    )
                    nc.vector.tensor_scalar_mul(
                        tt[:, :hr, W - 1], xt[:, :hr, W - 1], 3.0
                    )
                    # y = 3t + shift_h(t); last image row: y = 3t
                    yt = ypool.tile([128, STRIP, W], f32, tag="y")
                    if last:
                        nc.vector.scalar_tensor_tensor(
                            out=yt[:, 0 : STRIP - 1, :],
                            in0=tt[:, 0 : STRIP - 1, :],
                            scalar=3.0,
                            in1=tt[:, 1:STRIP, :],
                            op0=mult,
                            op1=add,
                        )
                        nc.vector.tensor_scalar_mul(
                            yt[:, STRIP - 1, :], tt[:, STRIP - 1, :], 3.0
                        )
                    else:
                        nc.vector.scalar_tensor_tensor(
                            out=yt[:],
                            in0=tt[:, 0:STRIP, :],
                            scalar=3.0,
                            in1=tt[:, 1 : STRIP + 1, :],
                            op0=mult,
                            op1=add,
                        )
                    nc.scalar.dma_start(
                        out=y[c0 : c0 + 128, h0 : h0 + STRIP, :], in_=yt[:]
                    )
    nc.compile()
    return nc


def kernel(x: np.ndarray) -> np.ndarray:
    x = np.ascontiguousarray(np.asarray(x, dtype=np.float32))
    assert x.shape == (B, C, H, W), x.shape
    if "nc" not in _cache:
        _cache["nc"] = _build()
    nc = _cache["nc"]
    in_maps = [{"x": np.ascontiguousarray(x[i])} for i in range(B)]
    res = run_bass_kernel_spmd(nc, in_maps, core_ids=list(range(N_CORES)))
    return np.stack([res.results[i]["y"] for i in range(N_CORES)], axis=0)


if __name__ == "__main__":
    rng = np.random.default_rng(0)
    x = rng.standard_normal((B, C, H, W), dtype=np.float32)
    y = kernel(x)
    xp = np.pad(x, ((0, 0), (0, 0), (0, 1), (0, 1)))
    ref = (
        9 * xp[:, :, :-1, :-1] + 3 * xp[:, :, :-1, 1:]
        + 3 * xp[:, :, 1:, :-1] + 1 * xp[:, :, 1:, 1:]
    ) / 16
    err = np.abs(y - ref).max()
    print("absmax err vs numpy stencil:", err, "scale:", np.abs(ref).max())
